# revision 7
# baseline (speedup 1.0000x reference)
"""GCN GraphConv (norm='both') on 8 Trainium2 NeuronCores.

Strategy (V3, scatter-free):
  - Output rows (dst nodes) sharded across 8 cores: core c owns rows
    [c*6250, (c+1)*6250), viewed as 49 blocks of 128 dst rows.
  - Every core computes the full projection h = feat @ W in bf16 (feat
    passed host-transposed as featT so PE contracts over in_feats),
    scales rows by outdeg^-1/2, and writes h (padded to 128 bf16 cols =
    256B rows) to local DRAM in a permuted row order rho(n) that makes
    the SBUF->DRAM writes dense.
  - Edge phase: per-core edges grouped by dst block; each block's edges
    padded to a whole number of 128-edge chunks (chunk budgets shared
    across cores so the program is SPMD-uniform).  dma_gather pulls the
    h[rho(src)] rows chunk-by-chunk into SBUF (token layout), a one-hot
    S[e,d] = (dstval[e] == d) is built on DVE, and PE accumulates
    psum[128 dst, 64] += S.T @ G per chunk.  No scatter -> no RMW races.
  - int16 gather-index limit handled by a low/high split at SPLIT with a
    base-offset view h_dram[SPLIT:].
  - Final per block: scale psum by indeg^-1/2, add bias, stage, one
    dense DMA out (host un-permutes the [p, g] row order).

Host does integer-only graph preprocessing (degree counts, edge
bucketing, index remapping).  bf16 is used for feat/W/h (kernel-internal
precision choice); accumulation is fp32 in PSUM.
"""

import sys

sys.path.insert(0, "/opt/trn_rl_repo")

import numpy as np
import ml_dtypes

import concourse.bacc as bacc
import concourse.bass as bass
import concourse.mybir as mybir
import concourse.tile as tile
from concourse.bass_utils import run_bass_kernel_spmd

F32 = mybir.dt.float32
BF16 = mybir.dt.bfloat16
I16 = mybir.dt.int16

N_NODES = 50000
N_FEAT = 256
N_OUT = 64
P = 128
NO_MATCH = 999.0  # dstval for pad slots; never equals iota 0..127


def _cfg_full():
    return dict(
        NPAD=50176,          # nodes padded to mult of 128 (C=392)
        OWN=6250,
        OWN_PAD=6272,        # 49 blocks of 128
        SPLIT=32768,
        SUP=16,              # h chunks per phase-1 superblock
        BLK_G=4,             # dst blocks per gather instruction group
        NF=N_FEAT,
        NO=N_OUT,
        NUM_DEV=8,
        CL=None,             # per-block lo chunk budgets (len 49)
        CH=None,             # per-block hi chunk budgets
    )


def build_nc(cfg, debug=False):
    NPAD, SPLIT = cfg["NPAD"], cfg["SPLIT"]
    OWN_PAD = cfg["OWN_PAD"]
    SUP, BLK_G = cfg["SUP"], cfg["BLK_G"]
    NF, NO = cfg["NF"], cfg["NO"]
    CL, CH = cfg["CL"], cfg["CH"]
    C = NPAD // P
    NBLK = OWN_PAD // P
    KC = NF // P
    assert len(CL) == NBLK and len(CH) == NBLK

    nc = bacc.Bacc(
        "TRN2",
        target_bir_lowering=False,
        debug=debug,
        num_devices=cfg["NUM_DEV"],
    )

    TL, TH = sum(CL) * P, sum(CH) * P  # total lo/hi gather slots

    featT = nc.dram_tensor("featT", [NF, NPAD], BF16, kind="ExternalInput")
    weight = nc.dram_tensor("weight", [NF, NO], BF16, kind="ExternalInput")
    bias_t = nc.dram_tensor("bias_t", [P, NO], F32, kind="ExternalInput")
    odeg = nc.dram_tensor("odeg", [P, C], F32, kind="ExternalInput")
    ideg = nc.dram_tensor("ideg", [P, NBLK], F32, kind="ExternalInput")
    iota_h = nc.dram_tensor("iota_h", [P, P], F32, kind="ExternalInput")
    gidxL = nc.dram_tensor("gidxL", [P, max(TL // 16, 1)], I16, kind="ExternalInput")
    gidxH = nc.dram_tensor("gidxH", [P, max(TH // 16, 1)], I16, kind="ExternalInput")
    # dvalsT[p, j] = dst-in-block of slot p in chunk j (chunk order: per
    # block, lo chunks then hi chunks, blocks ascending)
    TOTCK = sum(CL) + sum(CH)
    dvalsT = nc.dram_tensor("dvalsT", [P, TOTCK], F32, kind="ExternalInput")

    out = nc.dram_tensor("out", [P, NBLK * NO], F32, kind="ExternalOutput")

    h_dram = nc.dram_tensor("h_dram", [NPAD, P], BF16)
    h_view = h_dram.ap().rearrange("(p c) d -> p c d", p=P)

    with tile.TileContext(nc) as tc:
        with (
            tc.tile_pool(name="const", bufs=1) as cpool,
            tc.tile_pool(name="feat", bufs=2) as fpool,
            tc.tile_pool(name="hstage", bufs=2) as hpool,
            tc.tile_pool(name="psA", bufs=2, space="PSUM") as ppoolA,
            tc.tile_pool(name="psB", bufs=2, space="PSUM") as ppoolB,
            tc.tile_pool(name="gath", bufs=16) as gpool,
            tc.tile_pool(name="idx", bufs=2) as ipool,
            tc.tile_pool(name="onehot", bufs=6) as spool,
            tc.tile_pool(name="fin", bufs=1) as finpool,
        ):
            # ---- constants ----
            w_sb = []
            for k in range(KC):
                wk = cpool.tile([P, NO], BF16, tag=f"w{k}")
                nc.sync.dma_start(wk[:], weight[k * P:(k + 1) * P, :])
                w_sb.append(wk)
            bias_sb = cpool.tile([P, NO], F32, tag="bias")
            nc.sync.dma_start(bias_sb[:], bias_t[:])
            iota_sb = cpool.tile([P, P], F32, tag="iota")
            nc.sync.dma_start(iota_sb[:], iota_h[:])

            osc = cpool.tile([P, C], F32, tag="osc")
            nc.sync.dma_start(osc[:], odeg[:])
            nc.vector.tensor_scalar_max(osc[:], osc[:], 1.0)
            nc.scalar.activation(osc[:], osc[:], mybir.ActivationFunctionType.Sqrt)
            nc.vector.reciprocal(osc[:], osc[:])

            isc = cpool.tile([P, NBLK], F32, tag="isc")
            nc.sync.dma_start(isc[:], ideg[:])
            nc.vector.tensor_scalar_max(isc[:], isc[:], 1.0)
            nc.scalar.activation(isc[:], isc[:], mybir.ActivationFunctionType.Sqrt)
            nc.vector.reciprocal(isc[:], isc[:])

            # ---- phase 1: h = (feat @ W) * outdeg^-1/2, bf16, 128-col rows ----
            for c0 in range(0, C, SUP):
                sup = min(SUP, C - c0)
                fts = []
                for k in range(KC):
                    ft = fpool.tile([P, SUP * P], BF16, tag=f"ft{k}")
                    nc.sync.dma_start(
                        ft[:, : sup * P],
                        featT[k * P:(k + 1) * P, c0 * P:(c0 + sup) * P],
                    )
                    fts.append(ft)
                hst = hpool.tile([P, SUP * P], BF16, tag="hst")
                # pad cols NO..P of each row slot are never read by the edge
                # matmul, but the h-write DMA reads the whole tile
                nc.gpsimd.memset(hst[:], 0.0)
                for cc in range(sup):
                    hp = ppoolA.tile([P, NO], F32, tag="hp")
                    for k in range(KC):
                        nc.tensor.matmul(
                            hp[:],
                            fts[k][:, cc * P:(cc + 1) * P],
                            w_sb[k][:],
                            start=(k == 0),
                            stop=(k == KC - 1),
                        )
                    c = c0 + cc
                    # cols NO..P of each 128-col row slot stay junk; the
                    # edge matmul only reads cols 0..NO.
                    nc.vector.tensor_scalar_mul(
                        hst[:, cc * P:cc * P + NO], hp[:], osc[:, c:c + 1]
                    )
                nc.sync.dma_start(h_view[:, c0:c0 + sup, :], hst[:, : sup * P])

            # ---- phase 2: per dst block, gather + one-hot matmul reduce ----
            h_full = h_dram.ap()
            h_hi = h_dram.ap()[SPLIT:, :]
            ot = finpool.tile([P, NBLK * NO], F32, tag="out")

            offL = [0]
            offH = [0]
            for b in range(NBLK):
                offL.append(offL[-1] + CL[b])
                offH.append(offH[-1] + CH[b])
            offD = [offL[b] + offH[b] for b in range(NBLK + 1)]

            groups = [
                (g0, min(g0 + BLK_G, NBLK)) for g0 in range(0, NBLK, BLK_G)
            ]
            maxL = max((offL[g1] - offL[g0]) for g0, g1 in groups)
            maxH = max((offH[g1] - offH[g0]) for g0, g1 in groups)
            maxD = max((offD[g1] - offD[g0]) for g0, g1 in groups)

            # SWDGE descriptor ring caps a single dma_gather at <1024
            # descriptors; stay at <=7 chunks (896) per instruction.
            SEG = 7

            for g0, g1 in groups:
                ckL = offL[g1] - offL[g0]
                ckH = offH[g1] - offH[g0]
                nck = offD[g1] - offD[g0]

                gixL = gixH = None
                if ckL:
                    gixL = ipool.tile([P, max(maxL * 8, 8)], I16, tag="gixL")
                    nc.sync.dma_start(
                        gixL[:, : ckL * 8],
                        gidxL[:, offL[g0] * 8: offL[g1] * 8],
                    )
                if ckH:
                    gixH = ipool.tile([P, max(maxH * 8, 8)], I16, tag="gixH")
                    nc.sync.dma_start(
                        gixH[:, : ckH * 8],
                        gidxH[:, offH[g0] * 8: offH[g1] * 8],
                    )

                # segment gathers of <=SEG chunks; tiles[kind][seg] holds
                # group-local chunks [seg*SEG, seg*SEG+n)
                tiles = ([], [])
                for kind, (ck, gix, base_ap) in enumerate(
                    [(ckL, gixL, h_full), (ckH, gixH, h_hi)]
                ):
                    for s0 in range(0, ck, SEG):
                        n = min(SEG, ck - s0)
                        gt = gpool.tile([P, SEG, P], BF16, tag="gt")
                        nc.gpsimd.dma_gather(
                            gt[:, :n, :],
                            base_ap,
                            gix[:, s0 * 8:(s0 + n) * 8],
                            n * P,
                            n * P,
                            P,
                        )
                        tiles[kind].append(gt)

                dv = ipool.tile([P, max(maxD, 1)], F32, tag="dv")
                if nck:
                    nc.sync.dma_start(dv[:, :nck], dvalsT[:, offD[g0]:offD[g1]])

                for b in range(g0, g1):
                    tot = CL[b] + CH[b]
                    osl = slice(b * NO, (b + 1) * NO)
                    if tot == 0:
                        # empty block: bias only
                        nc.vector.tensor_copy(ot[:, osl], bias_sb[:])
                        continue
                    pb = ppoolB.tile([P, NO], F32, tag="pb")
                    j = 0
                    for kind in range(2):
                        cnt = CL[b] if kind == 0 else CH[b]
                        base = (offL[b] - offL[g0]) if kind == 0 else (offH[b] - offH[g0])
                        for cc in range(cnt):
                            dcol = offD[b] - offD[g0] + j
                            S = spool.tile([P, P], BF16, tag="S")
                            nc.vector.tensor_scalar(
                                S[:], iota_sb[:], dv[:, dcol:dcol + 1], None,
                                op0=mybir.AluOpType.is_equal,
                            )
                            gc = base + cc  # group-local chunk of this kind
                            gt = tiles[kind][gc // SEG]
                            nc.tensor.matmul(
                                pb[:],
                                S[:],
                                gt[:, gc % SEG, :NO],
                                start=(j == 0),
                                stop=(j == tot - 1),
                            )
                            j += 1
                    nc.vector.tensor_scalar_mul(ot[:, osl], pb[:], isc[:, b:b + 1])
                    nc.vector.tensor_tensor(
                        ot[:, osl], ot[:, osl], bias_sb[:], op=mybir.AluOpType.add
                    )

            nc.sync.dma_start(out.ap(), ot[:])

    nc.compile()
    return nc


def _wrap_stream(a):
    """flat [L] int array (L % 16 == 0) -> [128, L//16] int16 wrapped+replicated."""
    L = len(a)
    if L == 0:
        return np.zeros((P, 1), np.int16)
    w = a.reshape(L // 16, 16).T  # [16, L//16]
    return np.ascontiguousarray(np.tile(w, (8, 1)).astype(np.int16))


def _prep_host(feat, weight, bias, src, dst, cfg):
    NPAD, OWN, OWN_PAD = cfg["NPAD"], cfg["OWN"], cfg["OWN_PAD"]
    SPLIT = cfg["SPLIT"]
    NF = cfg["NF"]
    C = NPAD // P
    NBLK = OWN_PAD // P
    n = feat.shape[0]
    ncore = cfg["NUM_DEV"]

    src = np.asarray(src)
    dst = np.asarray(dst)
    # rho: node n -> h_dram row (n%128)*C + n//128
    rho_src = (src % P).astype(np.int64) * C + src // P

    outdeg = np.bincount(src, minlength=NPAD).astype(np.float32)
    outdeg[n:] = 1.0
    indeg = np.bincount(dst, minlength=ncore * OWN).astype(np.float32)

    featT = np.zeros((NF, NPAD), ml_dtypes.bfloat16)
    featT[:, :n] = np.asarray(feat, np.float32).T
    featT = np.ascontiguousarray(featT)

    odeg_r = np.ascontiguousarray(outdeg.reshape(C, P).T)
    bias_t = np.ascontiguousarray(
        np.tile(np.asarray(bias, np.float32)[None, :], (P, 1))
    )
    weight_b = np.ascontiguousarray(np.asarray(weight, np.float32).astype(ml_dtypes.bfloat16))
    iota_h = np.ascontiguousarray(
        np.tile(np.arange(P, dtype=np.float32)[None, :], (P, 1))
    )

    core_of = dst // OWN
    # per core, per block: lo/hi edge lists
    buckets = []
    for c in range(ncore):
        m = core_of == c
        g = rho_src[m]
        dl = (dst[m] - c * OWN).astype(np.int64)
        blk = dl // P
        d128 = dl % P
        lo = g < SPLIT
        per_blk = []
        for b in range(NBLK):
            mb = blk == b
            mbl = mb & lo
            mbh = mb & ~lo
            per_blk.append((g[mbl], d128[mbl], g[mbh] - SPLIT, d128[mbh]))
        buckets.append(per_blk)

    CL = [0] * NBLK
    CH = [0] * NBLK
    for c in range(ncore):
        for b in range(NBLK):
            gl, _, gh, _ = buckets[c][b]
            CL[b] = max(CL[b], (len(gl) + P - 1) // P)
            CH[b] = max(CH[b], (len(gh) + P - 1) // P)

    in_maps = []
    for c in range(ncore):
        gl_stream = np.zeros(sum(CL) * P, np.int64)
        gh_stream = np.zeros(sum(CH) * P, np.int64)
        dvals = np.full((sum(CL) + sum(CH), P), NO_MATCH, np.float32)
        oL = oH = oD = 0
        for b in range(NBLK):
            gl, dvl, gh, dvh = buckets[c][b]
            gl_stream[oL:oL + len(gl)] = gl
            gh_stream[oH:oH + len(gh)] = gh
            dvals.reshape(-1)[oD * P:oD * P + len(dvl)] = dvl
            oD += CL[b]
            dvals.reshape(-1)[oD * P:oD * P + len(dvh)] = dvh
            oD += CH[b]
            oL += CL[b] * P
            oH += CH[b] * P

        ideg_c = np.full(OWN_PAD, 1.0, np.float32)
        ideg_c[:OWN] = indeg[c * OWN:(c + 1) * OWN]
        in_maps.append(
            {
                "featT": featT,
                "weight": weight_b,
                "bias_t": bias_t,
                "odeg": odeg_r,
                "ideg": np.ascontiguousarray(ideg_c.reshape(NBLK, P).T),
                "iota_h": iota_h,
                "gidxL": _wrap_stream(gl_stream),
                "gidxH": _wrap_stream(gh_stream),
                "dvalsT": np.ascontiguousarray(dvals.T),
            }
        )
    return in_maps, CL, CH


_NC_CACHE = {}


def _get_nc(cfg):
    key = (tuple(cfg["CL"]), tuple(cfg["CH"]))
    if key not in _NC_CACHE:
        _NC_CACHE[key] = build_nc(cfg)
    return _NC_CACHE[key]


def kernel(feat, weight, bias, src, dst, _trace=False, _trace_kwargs=None):
    cfg = _cfg_full()
    in_maps, CL, CH = _prep_host(feat, weight, bias, src, dst, cfg)
    cfg["CL"], cfg["CH"] = CL, CH
    nc = _get_nc(cfg)
    res = run_bass_kernel_spmd(
        nc,
        in_maps,
        core_ids=list(range(cfg["NUM_DEV"])),
        trace=_trace,
        **(_trace_kwargs or {}),
    )
    OWN, NBLK, NO = cfg["OWN"], cfg["OWN_PAD"] // P, cfg["NO"]
    outs = []
    for c in range(cfg["NUM_DEV"]):
        arr = res.results[c]["out"].reshape(P, NBLK, NO)
        own = arr.transpose(1, 0, 2).reshape(NBLK * P, NO)[:OWN]
        outs.append(own)
    out = np.ascontiguousarray(np.concatenate(outs, axis=0).astype(np.float32))
    if _trace:
        return out, res
    return out


# revision 12
# speedup vs baseline: 2.5627x; 2.5627x over previous
"""GCN GraphConv (norm='both') on 8 Trainium2 NeuronCores.

Strategy (V4, scatter-free, sharded projection):
  - Output rows (dst nodes) sharded across 8 cores: core c owns rows
    [c*6250, (c+1)*6250), viewed as 49 blocks of 128 dst rows.
  - Projection phase sharded by node: core c computes h = feat @ W (bf16)
    for nodes [c*6272, (c+1)*6272), scaled by outdeg^-1/2, written to
    local DRAM as 256B rows (128 bf16 cols, top 64 junk) in a permuted
    order rho that makes the writes dense; then AllGather replicates the
    full h table to every core.
  - Edge phase: per-core edges grouped by dst block; each block's edges
    padded to whole 128-edge chunks (chunk budgets shared across cores so
    the program is SPMD-uniform).  dma_gather (4 SWDGE queues, <=7
    chunks per instruction to fit the descriptor ring) pulls h[rho(src)]
    rows into SBUF token layout; a one-hot S[e,d] = (dstval[e] == d) is
    built on DVE in bf16; PE accumulates psum[128 dst, 64] += S.T @ G
    per chunk.  No scatter -> no RMW races.
  - int16 gather-index limit handled by a low/high split at SPLIT with a
    base-offset view h_all[SPLIT:].
  - Final per block: scale psum by indeg^-1/2, add bias, stage, one
    dense DMA out (host un-permutes the [p, g] row order).

Host does integer-only graph preprocessing (degree counts, edge
bucketing, index remapping).  bf16 is used for feat/W/h (kernel-internal
precision choice); accumulation is fp32 in PSUM.
"""

import sys

sys.path.insert(0, "/opt/trn_rl_repo")

import numpy as np
import ml_dtypes

import concourse.bacc as bacc
import concourse.bass as bass
import concourse.mybir as mybir
import concourse.tile as tile
from concourse.bass_utils import run_bass_kernel_spmd

F32 = mybir.dt.float32
BF16 = mybir.dt.bfloat16
I16 = mybir.dt.int16

N_NODES = 50000
N_FEAT = 256
N_OUT = 64
P = 128
NO_MATCH = 999.0  # dstval for pad slots; never equals iota 0..127


def _cfg_full():
    return dict(
        SHARD=6272,          # phase-1 nodes per core (49*128); NPAD = 8*SHARD
        OWN=6250,
        OWN_PAD=6272,        # 49 blocks of 128
        SPLIT=32768,
        SUP=16,              # h chunks per phase-1 superblock
        BLK_G=4,             # dst blocks per idx-load group
        NF=N_FEAT,
        NO=N_OUT,
        NUM_DEV=8,
        CL=None,             # per-block lo chunk budgets (len 49)
        CH=None,             # per-block hi chunk budgets
    )


def build_nc(cfg, debug=False):
    SHARD, SPLIT = cfg["SHARD"], cfg["SPLIT"]
    OWN_PAD = cfg["OWN_PAD"]
    SUP, BLK_G = cfg["SUP"], cfg["BLK_G"]
    NF, NO = cfg["NF"], cfg["NO"]
    CL, CH = cfg["CL"], cfg["CH"]
    NDEV = cfg["NUM_DEV"]
    NPAD = SHARD * NDEV
    CLOC = SHARD // P    # local h chunks (phase 1)
    NBLK = OWN_PAD // P
    KC = NF // P
    assert len(CL) == NBLK and len(CH) == NBLK

    nc = bacc.Bacc(
        "TRN2",
        target_bir_lowering=False,
        debug=debug,
        num_devices=NDEV,
        num_swdge_queues=4,
    )

    TL, TH = sum(CL) * P, sum(CH) * P  # total lo/hi gather slots

    featT = nc.dram_tensor("featT", [NF, SHARD], BF16, kind="ExternalInput")
    weight = nc.dram_tensor("weight", [NF, NO], BF16, kind="ExternalInput")
    bias_t = nc.dram_tensor("bias_t", [P, NO], F32, kind="ExternalInput")
    odeg = nc.dram_tensor("odeg", [P, CLOC], F32, kind="ExternalInput")
    ideg = nc.dram_tensor("ideg", [P, NBLK], F32, kind="ExternalInput")
    iota_h = nc.dram_tensor("iota_h", [P, P], BF16, kind="ExternalInput")
    gidxL = nc.dram_tensor("gidxL", [P, max(TL // 16, 1)], I16, kind="ExternalInput")
    gidxH = nc.dram_tensor("gidxH", [P, max(TH // 16, 1)], I16, kind="ExternalInput")
    # dvalsT[p, j] = dst-in-block of slot p in chunk j (chunk order: per
    # block, lo chunks then hi chunks, blocks ascending)
    TOTCK = sum(CL) + sum(CH)
    dvalsT = nc.dram_tensor("dvalsT", [P, TOTCK], BF16, kind="ExternalInput")

    out = nc.dram_tensor("out", [P, NBLK * NO], F32, kind="ExternalOutput")

    h_own = nc.dram_tensor("h_own", [SHARD, P], BF16)
    h_all = nc.dram_tensor("h_all", [NPAD, P], BF16, addr_space="Shared")
    h_own_view = h_own.ap().rearrange("(p c) d -> p c d", p=P)

    with tile.TileContext(nc) as tc:
        with (
            tc.tile_pool(name="const", bufs=1) as cpool,
            tc.tile_pool(name="feat", bufs=2) as fpool,
            tc.tile_pool(name="hstage", bufs=2) as hpool,
            tc.tile_pool(name="psA", bufs=2, space="PSUM") as ppoolA,
            tc.tile_pool(name="psB", bufs=2, space="PSUM") as ppoolB,
            tc.tile_pool(name="gath", bufs=16) as gpool,
            tc.tile_pool(name="idx", bufs=2) as ipool,
            tc.tile_pool(name="onehot", bufs=6) as spool,
            tc.tile_pool(name="fin", bufs=1) as finpool,
        ):
            # ---- constants ----
            w_sb = []
            for k in range(KC):
                wk = cpool.tile([P, NO], BF16, tag=f"w{k}")
                nc.sync.dma_start(wk[:], weight[k * P:(k + 1) * P, :])
                w_sb.append(wk)
            bias_sb = cpool.tile([P, NO], F32, tag="bias")
            nc.sync.dma_start(bias_sb[:], bias_t[:])
            iota_sb = cpool.tile([P, P], BF16, tag="iota")
            nc.sync.dma_start(iota_sb[:], iota_h[:])

            osc = cpool.tile([P, CLOC], F32, tag="osc")
            nc.sync.dma_start(osc[:], odeg[:])
            nc.vector.tensor_scalar_max(osc[:], osc[:], 1.0)
            nc.scalar.activation(osc[:], osc[:], mybir.ActivationFunctionType.Sqrt)
            nc.vector.reciprocal(osc[:], osc[:])

            isc = cpool.tile([P, NBLK], F32, tag="isc")
            nc.sync.dma_start(isc[:], ideg[:])
            nc.vector.tensor_scalar_max(isc[:], isc[:], 1.0)
            nc.scalar.activation(isc[:], isc[:], mybir.ActivationFunctionType.Sqrt)
            nc.vector.reciprocal(isc[:], isc[:])

            # ---- phase 1: own-shard h = (feat @ W) * outdeg^-1/2 ----
            for c0 in range(0, CLOC, SUP):
                sup = min(SUP, CLOC - c0)
                fts = []
                for k in range(KC):
                    ft = fpool.tile([P, SUP * P], BF16, tag=f"ft{k}")
                    nc.sync.dma_start(
                        ft[:, : sup * P],
                        featT[k * P:(k + 1) * P, c0 * P:(c0 + sup) * P],
                    )
                    fts.append(ft)
                hst = hpool.tile([P, SUP * P], BF16, tag="hst")
                # pad cols NO..P of each row slot are never read by the edge
                # matmul, but the h-write DMA reads the whole tile
                nc.gpsimd.memset(hst[:], 0.0)
                for cc in range(sup):
                    hp = ppoolA.tile([P, NO], F32, tag="hp")
                    for k in range(KC):
                        nc.tensor.matmul(
                            hp[:],
                            fts[k][:, cc * P:(cc + 1) * P],
                            w_sb[k][:],
                            start=(k == 0),
                            stop=(k == KC - 1),
                        )
                    c = c0 + cc
                    nc.vector.tensor_scalar_mul(
                        hst[:, cc * P:cc * P + NO], hp[:], osc[:, c:c + 1]
                    )
                nc.sync.dma_start(h_own_view[:, c0:c0 + sup, :], hst[:, : sup * P])

            # ---- all-gather h across cores ----
            if NDEV > 1:
                nc.gpsimd.collective_compute(
                    "AllGather",
                    mybir.AluOpType.bypass,
                    replica_groups=[list(range(NDEV))],
                    ins=[h_own.ap()],
                    outs=[h_all.ap()],
                )
            else:
                nc.sync.dma_start(h_all.ap(), h_own.ap())

            # ---- phase 2: per dst block, gather + one-hot matmul reduce ----
            h_full = h_all.ap()
            h_hi = h_all.ap()[SPLIT:, :]
            ot = finpool.tile([P, NBLK * NO], F32, tag="out")

            offL = [0]
            offH = [0]
            for b in range(NBLK):
                offL.append(offL[-1] + CL[b])
                offH.append(offH[-1] + CH[b])
            offD = [offL[b] + offH[b] for b in range(NBLK + 1)]

            groups = [
                (g0, min(g0 + BLK_G, NBLK)) for g0 in range(0, NBLK, BLK_G)
            ]
            maxL = max((offL[g1] - offL[g0]) for g0, g1 in groups)
            maxH = max((offH[g1] - offH[g0]) for g0, g1 in groups)
            maxD = max((offD[g1] - offD[g0]) for g0, g1 in groups)

            # SWDGE descriptor ring caps a single dma_gather at <1024
            # descriptors; stay at <=7 chunks (896) per instruction, and
            # round-robin the 4 SWDGE queues so transfers overlap.
            SEG = 7
            qcnt = [0]

            for g0, g1 in groups:
                ckL = offL[g1] - offL[g0]
                ckH = offH[g1] - offH[g0]
                nck = offD[g1] - offD[g0]

                gixL = gixH = None
                if ckL:
                    gixL = ipool.tile([P, max(maxL * 8, 8)], I16, tag="gixL")
                    nc.sync.dma_start(
                        gixL[:, : ckL * 8],
                        gidxL[:, offL[g0] * 8: offL[g1] * 8],
                    )
                if ckH:
                    gixH = ipool.tile([P, max(maxH * 8, 8)], I16, tag="gixH")
                    nc.sync.dma_start(
                        gixH[:, : ckH * 8],
                        gidxH[:, offH[g0] * 8: offH[g1] * 8],
                    )

                # segment gathers of <=SEG chunks; tiles[kind][seg] holds
                # group-local chunks [seg*SEG, seg*SEG+n)
                tiles = ([], [])
                for kind, (ck, gix, base_ap) in enumerate(
                    [(ckL, gixL, h_full), (ckH, gixH, h_hi)]
                ):
                    for s0 in range(0, ck, SEG):
                        n = min(SEG, ck - s0)
                        gt = gpool.tile([P, SEG, P], BF16, tag="gt")
                        nc.gpsimd.dma_gather(
                            gt[:, :n, :],
                            base_ap,
                            gix[:, s0 * 8:(s0 + n) * 8],
                            n * P,
                            n * P,
                            P,
                            queue_num=qcnt[0] % 4,
                        )
                        qcnt[0] += 1
                        tiles[kind].append(gt)

                dv = ipool.tile([P, max(maxD, 1)], BF16, tag="dv")
                if nck:
                    nc.sync.dma_start(dv[:, :nck], dvalsT[:, offD[g0]:offD[g1]])

                for b in range(g0, g1):
                    tot = CL[b] + CH[b]
                    osl = slice(b * NO, (b + 1) * NO)
                    if tot == 0:
                        # empty block: bias only
                        nc.vector.tensor_copy(ot[:, osl], bias_sb[:])
                        continue
                    # (kind, group-local chunk) per block chunk, in dval order
                    chunks = [(0, (offL[b] - offL[g0]) + cc) for cc in range(CL[b])]
                    chunks += [(1, (offH[b] - offH[g0]) + cc) for cc in range(CH[b])]
                    pb = ppoolB.tile([P, NO], F32, tag="pb")
                    WS = 4
                    for w0 in range(0, tot, WS):
                        wn = min(WS, tot - w0)
                        d0 = offD[b] - offD[g0] + w0
                        Sw = spool.tile([P, WS * P], BF16, tag="S")
                        # one-hot for wn chunks in one op:
                        # Sw[p, w, d] = (iota[d] == dval[w][p])
                        nc.vector.tensor_tensor(
                            Sw[:, : wn * P].rearrange("p (w d) -> p w d", d=P),
                            iota_sb[:].rearrange("p (o d) -> p o d", o=1).broadcast_to(
                                [P, wn, P]
                            ),
                            dv[:, d0:d0 + wn].rearrange("p (w o) -> p w o", o=1).broadcast_to(
                                [P, wn, P]
                            ),
                            op=mybir.AluOpType.is_equal,
                        )
                        for jj in range(wn):
                            kind, gc = chunks[w0 + jj]
                            gt = tiles[kind][gc // SEG]
                            nc.tensor.matmul(
                                pb[:],
                                Sw[:, jj * P:(jj + 1) * P],
                                gt[:, gc % SEG, :NO],
                                start=(w0 + jj == 0),
                                stop=(w0 + jj == tot - 1),
                            )
                    nc.vector.tensor_scalar_mul(ot[:, osl], pb[:], isc[:, b:b + 1])
                    nc.vector.tensor_tensor(
                        ot[:, osl], ot[:, osl], bias_sb[:], op=mybir.AluOpType.add
                    )

            nc.sync.dma_start(out.ap(), ot[:])

    nc.compile()
    return nc


def _wrap_stream(a):
    """flat [L] int array (L % 16 == 0) -> [128, L//16] int16 wrapped+replicated."""
    L = len(a)
    if L == 0:
        return np.zeros((P, 1), np.int16)
    w = a.reshape(L // 16, 16).T  # [16, L//16]
    return np.ascontiguousarray(np.tile(w, (8, 1)).astype(np.int16))


def _prep_host(feat, weight, bias, src, dst, cfg):
    SHARD, OWN, OWN_PAD = cfg["SHARD"], cfg["OWN"], cfg["OWN_PAD"]
    SPLIT = cfg["SPLIT"]
    NF = cfg["NF"]
    NBLK = OWN_PAD // P
    CLOC = SHARD // P
    n = feat.shape[0]
    ncore = cfg["NUM_DEV"]
    NPAD = SHARD * ncore

    src = np.asarray(src)
    dst = np.asarray(dst)
    # rho: node n -> h_all row: shard s = n // SHARD, local m = n % SHARD,
    # row = s*SHARD + (m%128)*CLOC + m//128
    sh = src // SHARD
    m = src % SHARD
    rho_src = sh * SHARD + (m % P).astype(np.int64) * CLOC + m // P

    outdeg = np.bincount(src, minlength=NPAD).astype(np.float32)
    outdeg[n:] = 1.0
    indeg = np.bincount(dst, minlength=ncore * OWN).astype(np.float32)

    featT_full = np.zeros((NF, NPAD), ml_dtypes.bfloat16)
    featT_full[:, :n] = np.asarray(feat, np.float32).T

    bias_t = np.ascontiguousarray(
        np.tile(np.asarray(bias, np.float32)[None, :], (P, 1))
    )
    weight_b = np.ascontiguousarray(
        np.asarray(weight, np.float32).astype(ml_dtypes.bfloat16)
    )
    iota_h = np.ascontiguousarray(
        np.tile(np.arange(P, dtype=np.float32)[None, :], (P, 1)).astype(
            ml_dtypes.bfloat16
        )
    )

    core_of = dst // OWN
    buckets = []
    for c in range(ncore):
        msk = core_of == c
        g = rho_src[msk]
        dl = (dst[msk] - c * OWN).astype(np.int64)
        blk = dl // P
        d128 = dl % P
        lo = g < SPLIT
        per_blk = []
        for b in range(NBLK):
            mb = blk == b
            mbl = mb & lo
            mbh = mb & ~lo
            per_blk.append((g[mbl], d128[mbl], g[mbh] - SPLIT, d128[mbh]))
        buckets.append(per_blk)

    CL = [0] * NBLK
    CH = [0] * NBLK
    for c in range(ncore):
        for b in range(NBLK):
            gl, _, gh, _ = buckets[c][b]
            CL[b] = max(CL[b], (len(gl) + P - 1) // P)
            CH[b] = max(CH[b], (len(gh) + P - 1) // P)

    in_maps = []
    for c in range(ncore):
        gl_stream = np.zeros(sum(CL) * P, np.int64)
        gh_stream = np.zeros(sum(CH) * P, np.int64)
        dvals = np.full((sum(CL) + sum(CH), P), NO_MATCH, np.float32)
        oL = oH = oD = 0
        for b in range(NBLK):
            gl, dvl, gh, dvh = buckets[c][b]
            gl_stream[oL:oL + len(gl)] = gl
            gh_stream[oH:oH + len(gh)] = gh
            dvals.reshape(-1)[oD * P:oD * P + len(dvl)] = dvl
            oD += CL[b]
            dvals.reshape(-1)[oD * P:oD * P + len(dvh)] = dvh
            oD += CH[b]
            oL += CL[b] * P
            oH += CH[b] * P

        ideg_c = np.full(OWN_PAD, 1.0, np.float32)
        ideg_c[:OWN] = indeg[c * OWN:(c + 1) * OWN]

        # per-core phase-1 shard
        featT_c = np.ascontiguousarray(featT_full[:, c * SHARD:(c + 1) * SHARD])
        od = outdeg[c * SHARD:(c + 1) * SHARD]
        odeg_r = np.ascontiguousarray(od.reshape(CLOC, P).T)

        in_maps.append(
            {
                "featT": featT_c,
                "weight": weight_b,
                "bias_t": bias_t,
                "odeg": odeg_r,
                "ideg": np.ascontiguousarray(ideg_c.reshape(NBLK, P).T),
                "iota_h": iota_h,
                "gidxL": _wrap_stream(gl_stream),
                "gidxH": _wrap_stream(gh_stream),
"dvalsT": np.ascontiguousarray(dvals.T.astype(ml_dtypes.bfloat16)),
            }
        )
    return in_maps, CL, CH


_NC_CACHE = {}


def _get_nc(cfg):
    key = (tuple(cfg["CL"]), tuple(cfg["CH"]))
    if key not in _NC_CACHE:
        _NC_CACHE[key] = build_nc(cfg)
    return _NC_CACHE[key]


def kernel(feat, weight, bias, src, dst, _trace=False, _trace_kwargs=None):
    cfg = _cfg_full()
    in_maps, CL, CH = _prep_host(feat, weight, bias, src, dst, cfg)
    cfg["CL"], cfg["CH"] = CL, CH
    nc = _get_nc(cfg)
    res = run_bass_kernel_spmd(
        nc,
        in_maps,
        core_ids=list(range(cfg["NUM_DEV"])),
        trace=_trace,
        **(_trace_kwargs or {}),
    )
    OWN, NBLK, NO = cfg["OWN"], cfg["OWN_PAD"] // P, cfg["NO"]
    outs = []
    for c in range(cfg["NUM_DEV"]):
        arr = res.results[c]["out"].reshape(P, NBLK, NO)
        own = arr.transpose(1, 0, 2).reshape(NBLK * P, NO)[:OWN]
        outs.append(own)
    out = np.ascontiguousarray(np.concatenate(outs, axis=0).astype(np.float32))
    if _trace:
        return out, res
    return out
